# revision 3
# baseline (speedup 1.0000x reference)
"""PointPillar NMS pre-selection kernel for 8x Trainium2 NeuronCores.

Computes, for cls_preds (8M, 3) and box_preds (8M, 7):
    scores = max(cls_preds, axis=-1)
    top_scores, idx = top_k(scores, 4096)   (jax tie-break: score desc, idx asc)
    returns (top_scores, box_preds[idx], argmax(cls_preds[idx])+1)

Strategy: data-parallel over the box dimension. Each of the 8 cores DMA-streams
its 1M-row cls shard, computes row maxes on the Vector engine, and runs the
GPSIMD `topk` library instruction to extract the top-256 (value, index) pairs
per 50176-element "token" (20 tokens/core). The 8*20*256 = 40960 candidates are
merged host-side with exact jax-equivalent tie-breaking; box rows and labels
are gathered host-side for the final 4096 indices only. A per-token soundness
check proves the candidate pool covers the exact global top-k; a numpy fallback
guards the (never observed) failure path.
"""

import numpy as np

N = 8_000_000
NCLS = 3
BOX_DIM = 7
K = 4096
NCORES = 8
N_CORE = N // NCORES  # 1,000,000 rows per core

V = 50_176            # elements per topk token (must be >50000 and %128==0)
C = V // 16           # 3136 columns per partition
KTOK = 256            # top-k per token (hardware fixed)
CALLS = [8, 8, 4]     # tokens per gpsimd.topk call (<=8 each); 20 tokens total
TOK_TOTAL = sum(CALLS)
N_PAD = TOK_TOTAL * V          # 1,003,520 rows per core after padding
CALL_ROWS = 8 * V              # rows covered by a full 8-token call
SUB = 4                        # DMA sub-chunks per call
W = 3 * C // SUB               # floats per partition per sub-chunk (2352)
CSUB = C // SUB                # scores per partition per sub-chunk (784)
PAD_VAL = -3.0e38

_CACHE = {}


def _build_program():
    import concourse.bacc as bacc
    import concourse.mybir as mybir
    import concourse.tile as tile

    f32 = mybir.dt.float32
    u32 = mybir.dt.uint32

    nc = bacc.Bacc("TRN2", target_bir_lowering=False, debug=False)
    cls_in = nc.dram_tensor("cls", [N_PAD * NCLS], f32, kind="ExternalInput")
    cands = nc.dram_tensor("cands", [len(CALLS) * 128 * 32], u32, kind="ExternalOutput")

    # topk's builder requires raw SBUF tensor handles (not pool tiles), so the
    # score/candidate buffers are allocated per call outside the pools.
    scores_t = [
        nc.alloc_sbuf_tensor(f"scores{k}", [128, C], f32) for k in range(len(CALLS))
    ]
    out_ts = [
        nc.alloc_sbuf_tensor(f"cand{k}", [128, 32], u32) for k in range(len(CALLS))
    ]

    with tile.TileContext(nc) as tc:
        with tc.tile_pool(name="raw", bufs=6) as rawp:
            cls_ap = cls_in.ap()
            cands_ap = cands.ap()
            for k, ntok in enumerate(CALLS):
                P = ntok * 16
                base = k * 128 * 3 * C  # float offset of call k's rows
                call_view = cls_ap[base : base + P * 3 * C].rearrange(
                    "(p f) -> p f", p=P
                )
                scores = scores_t[k].ap()
                for s in range(SUB):
                    raw = rawp.tile([128, W], f32)
                    nc.sync.dma_start(
                        out=raw[:P, :], in_=call_view[:, s * W : (s + 1) * W]
                    )
                    nc.vector.tensor_reduce(
                        out=scores[:P, s * CSUB : (s + 1) * CSUB],
                        in_=raw[:P, :].rearrange("p (a b) -> p a b", b=NCLS),
                        axis=mybir.AxisListType.X,
                        op=mybir.AluOpType.max,
                    )
                out_t = out_ts[k].ap()
                nc.gpsimd.topk(
                    out_ap=out_t[:P, :],
                    in_ap=scores[:P, :],
                    tokens=ntok,
                    vocab_size=V,
                    k=KTOK,
                )
                dst = cands_ap[k * 4096 : k * 4096 + P * 32].rearrange(
                    "(p f) -> p f", p=P
                )
                nc.sync.dma_start(out=dst, in_=out_t[:P, :])

    nc.compile()
    return nc


def _get_program():
    if "nc" not in _CACHE:
        _CACHE["nc"] = _build_program()
    return _CACHE["nc"]


def _make_in_maps(cls_preds: np.ndarray) -> list[dict]:
    in_maps = []
    for c in range(NCORES):
        shard = np.full((N_PAD, NCLS), PAD_VAL, dtype=np.float32)
        shard[:N_CORE] = cls_preds[c * N_CORE : (c + 1) * N_CORE]
        in_maps.append({"cls": shard.reshape(-1)})
    return in_maps


def _decode_candidates(results: list[dict]):
    """Decode per-core topk outputs into flat (scores, global_idx) candidate
    arrays plus the per-token minimum returned value (for the soundness
    check)."""
    all_scores, all_gidx, token_mins = [], [], []
    for c, res in enumerate(results):
        arr = np.ascontiguousarray(res["cands"]).reshape(len(CALLS), 128, 32)
        for k, ntok in enumerate(CALLS):
            for t in range(ntok):
                rows = arr[k, 16 * t : 16 * (t + 1), :]
                vals = rows[:, :16].reshape(KTOK).view(np.float32)
                idxs = rows[:, 16:].reshape(KTOK).astype(np.int64)
                r = k * CALL_ROWS + t * V + idxs  # row within the padded shard
                valid = r < N_CORE
                vals, r = vals[valid], r[valid]
                token_mins.append(vals.min() if vals.size else np.float32(PAD_VAL))
                all_scores.append(vals)
                all_gidx.append(c * N_CORE + r)
    return (
        np.concatenate(all_scores),
        np.concatenate(all_gidx),
        np.array(token_mins, dtype=np.float32),
    )


def _exact_topk_from_candidates(scores, gidx):
    """jax.lax.top_k ordering: descending value, ties broken by ascending
    index."""
    order = np.lexsort((gidx, -scores.astype(np.float64)))[:K]
    return scores[order], gidx[order]


def _fallback_full_topk(cls_preds):
    scores = cls_preds.max(axis=1)
    part = np.argpartition(-scores, K + 1024)[: K + 1024]
    return _exact_topk_from_candidates(scores[part], part.astype(np.int64))


def kernel(cls_preds: np.ndarray, box_preds: np.ndarray):
    from concourse import bass_utils

    cls_preds = np.asarray(cls_preds, dtype=np.float32)
    box_preds = np.asarray(box_preds, dtype=np.float32)

    nc = _get_program()
    in_maps = _make_in_maps(cls_preds)
    res = bass_utils.run_bass_kernel_spmd(nc, in_maps, core_ids=list(range(NCORES)))

    scores, gidx, token_mins = _decode_candidates(res.results)
    top_scores, top_idx = _exact_topk_from_candidates(scores, gidx)

    # Soundness: every element NOT returned by a token is <= that token's
    # minimum returned value. If that minimum is strictly below the merged
    # 4096th score, nothing outside the candidate pool can displace or tie
    # into the top-k. Otherwise fall back to an exact host computation.
    t_hat = top_scores[-1]
    if not (token_mins < t_hat).all():
        top_scores, top_idx = _fallback_full_topk(cls_preds)

    top_boxes = box_preds[top_idx]
    top_labels = (cls_preds[top_idx].argmax(axis=1) + 1).astype(np.int32)
    return top_scores.astype(np.float32), top_boxes, top_labels


# revision 4
# speedup vs baseline: 6.1023x; 6.1023x over previous
"""PointPillar NMS pre-selection kernel for 8x Trainium2 NeuronCores.

Computes, for cls_preds (8M, 3) and box_preds (8M, 7):
    scores = max(cls_preds, axis=-1)
    top_scores, idx = top_k(scores, 4096)   (jax tie-break: score desc, idx asc)
    returns (top_scores, box_preds[idx], argmax(cls_preds[idx])+1)

Strategy: data-parallel over the box dimension (distributed top-k reduction).
Each of the 8 cores DMA-streams its 1M-row cls shard and reduces every block of
16 rows to the block's score maximum (one fused VectorE max-reduce over 48
contiguous floats per block — the row max over 3 classes and the 16-row block
max in a single instruction). That is the entire device program: it reads the
full 96MB of cls data at DMA roofline and emits 8 x 62592 block maxima.

The host merge is exact: every element of the global top-4096 lives in a block
whose max is >= the 4096th-largest block maximum, so gathering the top-4096
blocks (ties included) is a provably sufficient candidate set. The host
rescores those ~4096 blocks (~65K rows), runs the exact jax-equivalent
tie-broken top-k, and gathers boxes/labels for the final 4096 indices only.
box_preds never touches the device.
"""

import numpy as np

N = 8_000_000
NCLS = 3
BOX_DIM = 7
K = 4096
NCORES = 8
N_CORE = N // NCORES      # 1,000,000 rows per core

BLK = 16                  # rows per block-max
ROWS_P = 7824             # rows per partition (128 * 7824 = 1,001,472 >= 1e6)
BPP = ROWS_P // BLK       # 489 blocks per partition
N_PAD = 128 * ROWS_P      # padded rows per core
# chunk widths in blocks per partition (sum = BPP); ~1.17MB DMA chunks
CHUNKS = [49] * 9 + [48]
PAD_VAL = -3.0e38

_CACHE = {}


def _build_program():
    import concourse.bacc as bacc
    import concourse.mybir as mybir
    import concourse.tile as tile

    f32 = mybir.dt.float32

    nc = bacc.Bacc("TRN2", target_bir_lowering=False, debug=False)
    cls_in = nc.dram_tensor("cls", [N_PAD * NCLS], f32, kind="ExternalInput")
    bmax_out = nc.dram_tensor("bmax", [128 * BPP], f32, kind="ExternalOutput")

    bmax_sb = nc.alloc_sbuf_tensor("bmax_sb", [128, BPP], f32)

    with tile.TileContext(nc) as tc:
        with tc.tile_pool(name="raw", bufs=6) as rawp:
            src = cls_in.ap().rearrange("(p f) -> p f", p=128)  # [128, 23472]
            bo = 0
            for w in CHUNKS:
                fw = w * BLK * NCLS  # floats per partition this chunk
                raw = rawp.tile([128, fw], f32, name=f"raw{bo}", tag="raw")
                nc.sync.dma_start(
                    out=raw[:, :], in_=src[:, bo * BLK * NCLS : bo * BLK * NCLS + fw]
                )
                nc.vector.tensor_reduce(
                    out=bmax_sb.ap()[:, bo : bo + w],
                    in_=raw[:, :].rearrange("p (a b) -> p a b", b=BLK * NCLS),
                    axis=mybir.AxisListType.X,
                    op=mybir.AluOpType.max,
                )
                bo += w
            nc.sync.dma_start(
                out=bmax_out.ap().rearrange("(p f) -> p f", p=128), in_=bmax_sb.ap()
            )

    nc.compile()
    return nc


def _get_program():
    if "nc" not in _CACHE:
        _CACHE["nc"] = _build_program()
    return _CACHE["nc"]


def _make_in_maps(cls_preds: np.ndarray) -> list[dict]:
    in_maps = []
    for c in range(NCORES):
        shard = np.full((N_PAD, NCLS), PAD_VAL, dtype=np.float32)
        shard[:N_CORE] = cls_preds[c * N_CORE : (c + 1) * N_CORE]
        in_maps.append({"cls": shard.reshape(-1)})
    return in_maps


def _merge_host(results: list[dict], cls_preds: np.ndarray):
    """Exact top-K from per-core block maxima.

    Soundness: let v* be the K-th largest block max. The top-K block maxima
    are K distinct elements each >= v*, so the K-th largest element t* >= v*.
    Any element >= t* lies in a block whose max >= t* >= v*, i.e. in the
    selected set {blocks : bmax >= v*}.
    """
    bm = np.stack([r["bmax"] for r in results])  # [8, 128*BPP]
    flat = bm.reshape(-1)
    part = np.argpartition(-flat, K - 1)
    vstar = flat[part[K - 1]]
    sel = np.flatnonzero(flat >= vstar)  # block ids, ties included

    # decode block id -> 16 global row indices
    c, rem = np.divmod(sel, 128 * BPP)
    p, b = np.divmod(rem, BPP)
    r0 = p.astype(np.int64) * ROWS_P + b.astype(np.int64) * BLK
    rows = (c.astype(np.int64) * N_CORE)[:, None] + r0[:, None] + np.arange(BLK)
    valid_block = r0 + BLK <= N_CORE  # pad blocks are never selected, but be safe
    rows = rows[valid_block].reshape(-1)

    scores = cls_preds[rows].max(axis=1)
    order = np.lexsort((rows, -scores.astype(np.float64)))[:K]
    return scores[order], rows[order]


def kernel(cls_preds: np.ndarray, box_preds: np.ndarray):
    from concourse import bass_utils

    cls_preds = np.asarray(cls_preds, dtype=np.float32)
    box_preds = np.asarray(box_preds, dtype=np.float32)

    nc = _get_program()
    in_maps = _make_in_maps(cls_preds)
    res = bass_utils.run_bass_kernel_spmd(nc, in_maps, core_ids=list(range(NCORES)))

    top_scores, top_idx = _merge_host(res.results, cls_preds)

    top_boxes = box_preds[top_idx]
    top_labels = (cls_preds[top_idx].argmax(axis=1) + 1).astype(np.int32)
    return top_scores.astype(np.float32), top_boxes, top_labels
